# revision 26
# baseline (speedup 1.0000x reference)
"""Self-contained TRN2 Bass kernel for nn_Attention_26044681683510.

Multi-head attention (B=2, N=2048, C=1024, H=16, rotary, softmax, out-proj),
sharded over 8 NeuronCores as (batch b, head-group hg of 4 heads):
data-parallel on batch, tensor-parallel on heads (column-parallel QKV,
row-parallel out-proj with host-side partial-sum reduction).

v2 dataflow (per core):
  A: Q^T/K^T projections in fp32r (accuracy-critical path) with even/odd
     channel split, rotary on DVE/Pool, DMA rearrange into per-head layouts.
     V projection in fp32r -> stored bf16 with an appended ones column
     (fused softmax-denominator row-sum). All of x and the weights are
     prefetched up front (x both halves resident) so the PE never waits on
     the second-half load.
  B: scores_T = kfp[h].T @ qf (fp32r) -> PSUM; exp on ACT (scale=1/8) ->
     bf16 SBUF; PV = [v|1].T @ exp in bf16 accumulated over k-tiles in PSUM.
     Normalization reads PSUM directly (fast reciprocal + gpsimd
     partition_broadcast + DVE multiply) -> bf16 attn.
  C: partial out-proj attn^T.T @ wpT in bf16 -> (seq, 1024) fp32 PSUM ->
     bf16 partials DMA'd out; host sums the 4 head-group partials per batch
     in fp32 and adds the bias.

QK path stays fp32r (11-bit mantissa) end-to-end; V/softmax/out-proj run in
bf16 (fp32 accumulation) -> ~4.5e-3 scale-relative error vs fp32 reference.
"""
import sys

for _p in ("/opt/trn_rl_repo",):
    if _p not in sys.path:
        sys.path.insert(0, _p)

import numpy as np
import ml_dtypes
import concourse.bass as bass
import concourse.mybir as mybir
import concourse.tile as tile
from concourse import bacc

F32 = mybir.dt.float32
F32R = mybir.dt.float32r
BF16 = mybir.dt.bfloat16
AFT = mybir.ActivationFunctionType

B, N, C, H = 2, 2048, 1024, 16
D = C // H
HPG = 4               # heads per core
CL = HPG * D          # 256 local channels
NK = C // 128         # 8 contraction tiles
NSEQ = N // 128       # 16 seq tiles
VW2 = 72              # padded per-head v block: [v(64) | ones(1) | pad(7)]
NCORES = 8


def _round_fp32r(a):
    b = np.ascontiguousarray(a, dtype=np.float32).view(np.uint32)
    mask = np.uint32((1 << 12) - 1)
    add = (b >> 12) & np.uint32(1)
    return ((b + mask // 2 + add) & ~mask).view(np.float32)


def _bf16(a):
    return np.ascontiguousarray(a, dtype=np.float32).astype(ml_dtypes.bfloat16)


def _pack_w(a):
    """[K*128, M] -> [128, K*M]: k-tile k at columns M*k (single-DMA layout)."""
    K, M = a.shape[0] // 128, a.shape[1]
    return _bf16(a.reshape(K, 128, M).transpose(1, 0, 2).reshape(128, K * M))


def _emit_body(tc, nc, t, uid=""):
    with tc.tile_pool(name=f"cst{uid}", bufs=1) as cst, \
         tc.tile_pool(name=f"qk{uid}", bufs=1) as qkp:
        wp_t = cst.tile([128, 2048], BF16, name=f"wp{uid}")
        nc.sync.dma_start(wp_t[:], t["wpT"][:, :])

        # packed q/k: pair i at columns 2048i; within a pair, head 2i on
        # partitions 0-63 and head 2i+1 on partitions 64-127
        qf_a = qkp.tile([128, 4096], BF16, name=f"qfa{uid}")
        kf_a = qkp.tile([128, 4096], BF16, name=f"kfa{uid}")
        v_sb = [qkp.tile([128, HPG * VW2], BF16, name=f"vsb{m}{uid}") for m in range(NSEQ)]
        attn_f = [qkp.tile([128, 2048], BF16, name=f"af{i}{uid}") for i in range(2)]
        for m in range(NSEQ):
            nc.gpsimd.memset(
                v_sb[m][:].rearrange("p (h w) -> p h w", h=HPG)[:, :, D:VW2], 1.0)

        def q_rows(h):
            return 64 * (h % 2)

        # ---------------- Phase A: QK projections + rotary ----------------
        xp_cm = tc.tile_pool(name=f"xa{uid}", bufs=1)
        xp = xp_cm.__enter__()
        wqkv_cm = tc.tile_pool(name=f"wqkv{uid}", bufs=1)
        wqkv = wqkv_cm.__enter__()
        csp_cm = tc.tile_pool(name=f"cs{uid}", bufs=1)
        csp = csp_cm.__enter__()
        rtp_cm = tc.tile_pool(name=f"rtmp{uid}", bufs=1)
        rtp = rtp_cm.__enter__()
        psA_cm = tc.tile_pool(name=f"psA{uid}", bufs=2, space="PSUM")
        psA = psA_cm.__enter__()

        # host-packed weights: one [128, 8*256] tile per projection, k-tile k
        # at columns 256k (single DMA each)
        wq_t = wqkv.tile([128, NK * 256], BF16, name=f"wq{uid}")
        wk_t = wqkv.tile([128, NK * 256], BF16, name=f"wk{uid}")
        wv_t = wqkv.tile([128, NK * 256], BF16, name=f"wv{uid}")
        # x^T k-tiles, full token range: one [128, 2048] tile per k
        xh = [xp.tile([128, 2048], BF16, name=f"x{k}{uid}") for k in range(NK)]
        cos_f = csp.tile([128, 2048], BF16, name=f"cos{uid}")
        sin_f = csp.tile([128, 2048], BF16, name=f"sin{uid}")
        # issue order ~= arrival order
        nc.sync.dma_start(wq_t[:], t["wqT"][:, :])
        nc.sync.dma_start(wk_t[:], t["wkT"][:, :])
        for k in range(NK):
            nc.sync.dma_start(xh[k][:], t["xT"][128*k:128*(k+1), :])
        nc.sync.dma_start(cos_f[:], t["cosr"][:, :])
        nc.sync.dma_start(sin_f[:], t["sinr"][:, :])
        nc.sync.dma_start(wv_t[:], t["wvT"][:, :])

        for half in range(2):
            rot_t = {(pre, eo): rtp.tile([128, 1024], BF16,
                                         name=f"r{pre}{eo}{half}{uid}",
                                         tag=f"r{pre}{eo}", bufs=2)
                     for pre in ("q", "k") for eo in ("e", "o")}
            for c in range(2):
                gof = 1024 * half + 512 * c
                pp = {}
                for nm, wt, sel in (("qe", wq_t, 0), ("qo", wq_t, 1),
                                    ("ke", wk_t, 0), ("ko", wk_t, 1)):
                    ps = psA.tile([128, 512], F32, name=f"ps_{nm}_{half}_{c}{uid}", tag=f"ps{nm}")
                    for k in range(NK):
                        nc.tensor.matmul(ps[:], wt[:, 256*k+128*sel:256*k+128*(sel+1)],
                                         xh[k][:, gof:gof+512],
                                         start=(k == 0), stop=(k == NK - 1))
                    pp[nm] = ps
                cs = cos_f[:, gof:gof+512]
                sn = sin_f[:, gof:gof+512]
                for pre in ("q", "k"):
                    e_ps, o_ps = pp[pre + "e"], pp[pre + "o"]
                    ta = rtp.tile([128, 512], F32, name=f"ta{pre}{half}{c}{uid}", tag="ta", bufs=2)
                    tb = rtp.tile([128, 512], F32, name=f"tb{pre}{half}{c}{uid}", tag="tb", bufs=2)
                    nc.vector.tensor_mul(ta[:], e_ps[:], cs)
                    nc.vector.tensor_mul(tb[:], o_ps[:], sn)
                    tc2 = rtp.tile([128, 512], F32, name=f"tc{pre}{half}{c}{uid}", tag="tc", bufs=2)
                    td = rtp.tile([128, 512], F32, name=f"td{pre}{half}{c}{uid}", tag="td", bufs=2)
                    nc.vector.tensor_mul(tc2[:], e_ps[:], sn)
                    nc.vector.tensor_mul(td[:], o_ps[:], cs)
                    # full-width rotate at 2x bf16 DVE rate into a per-half
                    # accumulator tile; scatter once per (pre, half) below
                    re = rot_t[(pre, "e")]
                    ro_ = rot_t[(pre, "o")]
                    nc.vector.tensor_sub(re[:, 512*c:512*(c+1)], ta[:], tb[:])
                    nc.vector.tensor_add(ro_[:, 512*c:512*(c+1)], tc2[:], td[:])
                    if c == 1:
                        # plain 2D scatter DMAs: src rows 32h -> dst tile rows
                        # 64*(h%2) (+32 for ro), pair h//2 column block
                        dstt = qf_a if pre == "q" else kf_a
                        hof2 = 1024 * half
                        for h in range(HPG):
                            pc = 2048 * (h // 2) + hof2
                            rw = 64 * (h % 2)
                            nc.sync.dma_start(dstt[rw:rw+32, pc:pc+1024],
                                              re[32*h:32*h+32, :])
                            nc.sync.dma_start(dstt[rw+32:rw+64, pc:pc+1024],
                                              ro_[32*h:32*h+32, :])

        for cm in (psA_cm, rtp_cm, csp_cm):
            cm.__exit__(None, None, None)

        # ------- Phase AB: V projection with head-0 scores/exp overlapped ----
        exp_cm = tc.tile_pool(name=f"ex{uid}", bufs=12, side="right")
        exp_p = exp_cm.__enter__()
        nrp_cm = tc.tile_pool(name=f"nr{uid}", bufs=1, side="right")
        nrp = nrp_cm.__enter__()
        scp_cm = tc.tile_pool(name=f"scp{uid}", bufs=1, space="PSUM", side="right")
        scp = scp_cm.__enter__()
        psV_cm = tc.tile_pool(name=f"psV{uid}", bufs=2, space="PSUM")
        psV = psV_cm.__enter__()

        ex_t = {}

        def emit_scores_exp(h, i, kt):
            ex = exp_p.tile([128, 1024], BF16, name=f"ex{h}_{i}_{kt}{uid}", tag="ex")
            sc = scp.tile([128, 1024], F32, name=f"sc{h}_{i}_{kt}{uid}", tag="sc", bufs=2)
            ro = q_rows(h)
            po = 2048 * (h // 2)
            hof = 1024 * i
            for cc in range(2):
                nc.tensor.matmul(sc[:, 512*cc:512*(cc+1)],
                                 kf_a[ro:ro+64, po+128*kt:po+128*(kt+1)],
                                 qf_a[ro:ro+64, po+hof+512*cc:po+hof+512*(cc+1)],
                                 start=True, stop=True)
            nc.scalar.activation(ex[:], sc[:], AFT.Exp, scale=0.125)
            ex_t[(h, i, kt)] = ex

        def emit_pv(h, i, kt, pv):
            ex = ex_t.pop((h, i, kt))
            for cc in range(2):
                nc.tensor.matmul(pv[:, 512*cc:512*(cc+1)],
                                 v_sb[kt][:, VW2*h:VW2*h+D+1],
                                 ex[:, 512*cc:512*(cc+1)],
                                 start=(kt == 0), stop=(kt == NSEQ - 1))

        def emit_norm(h, i, pv):
            ti, off = h // 2, 64 * (h % 2)
            hof = 1024 * i
            rs = nrp.tile([1, 1024], F32, name=f"rs{h}_{i}{uid}", tag="rs")
            # custom-DVE ops misread PSUM on HW: bounce through SBUF
            nc.vector.tensor_copy(rs[:], pv[64:65, :])
            rsr = nrp.tile([1, 1024], F32, name=f"rsr{h}_{i}{uid}", tag="rsr")
            nc.vector.reciprocal_approx_fast(rsr[:], rs[:])
            bc = nrp.tile([64, 1024], F32, name=f"bc{h}_{i}{uid}", tag="bc")
            nc.gpsimd.partition_broadcast(bc[:], rsr[:], channels=64)
            nc.vector.tensor_mul(attn_f[ti][off:off+64, hof:hof+1024],
                                 pv[0:64, :], bc[:])

        for mt in range(NSEQ):
            psv = psV.tile([128, 256], F32, name=f"psv{mt}{uid}", tag="psv")
            for k in range(NK):
                nc.tensor.matmul(psv[:], xh[k][:, 128*mt:128*(mt+1)],
                                 wv_t[:, 256*k:256*(k+1)],
                                 start=(k == 0), stop=(k == NK - 1))
            vdst = v_sb[mt][:].rearrange("p (h w) -> p h w", h=HPG)[:, :, 0:D]
            vsrc = psv[:].rearrange("p (h d) -> p h d", h=HPG)
            nc.vector.tensor_copy(vdst, vsrc)
            emit_scores_exp(0, 0, mt)

        psV_cm.__exit__(None, None, None)
        wqkv_cm.__exit__(None, None, None)
        xp_cm.__exit__(None, None, None)

        # ---------------- Phase B: attention ----------------
        pvp_cm = tc.tile_pool(name=f"pvp{uid}", bufs=1, space="PSUM")
        pvp = pvp_cm.__enter__()

        for h in range(HPG):
            for i in range(2):
                pv = pvp.tile([65, 1024], F32, name=f"pv{h}_{i}{uid}", tag="pv", bufs=2)
                for kt in range(NSEQ):
                    if (h, i) != (0, 0):
                        emit_scores_exp(h, i, kt)
                    emit_pv(h, i, kt, pv)
                emit_norm(h, i, pv)

        for cm in (pvp_cm, scp_cm, nrp_cm, exp_cm):
            cm.__exit__(None, None, None)

        # ---------------- Phase C: partial out-proj ----------------
        with tc.tile_pool(name=f"pop{uid}", bufs=2, space="PSUM") as pop, \
             tc.tile_pool(name=f"ob{uid}", bufs=2) as obp:
            for mt in range(NSEQ):
                po = pop.tile([128, 1024], F32, name=f"po{mt}{uid}", tag="po")
                for k in range(2):
                    for nchunk in range(2):
                        nc.tensor.matmul(po[:, 512*nchunk:512*(nchunk+1)],
                                         attn_f[k][:, 128*mt:128*(mt+1)],
                                         wp_t[:, 1024*k+512*nchunk:1024*k+512*(nchunk+1)],
                                         start=(k == 0), stop=(k == 1))
                ob = obp.tile([128, 1024], BF16, name=f"ob{mt}{uid}", tag=f"ob{mt % 2}", bufs=2)
                if mt % 2 == 0:
                    nc.vector.tensor_copy(ob[:], po[:])
                else:
                    nc.scalar.activation(ob[:], po[:], AFT.Copy)
                nc.sync.dma_start(t["outp"][128*mt:128*(mt+1), :], ob[:])


def _build_nc(rep=1, num_devices=NCORES):
    nc = bacc.Bacc("TRN2", target_bir_lowering=False, debug=False,
                   num_devices=num_devices)
    t = {}
    t["xT"] = nc.dram_tensor("xT", [C, N], BF16, kind="ExternalInput").ap()
    t["wqT"] = nc.dram_tensor("wqT", [128, NK * 256], BF16, kind="ExternalInput").ap()
    t["wkT"] = nc.dram_tensor("wkT", [128, NK * 256], BF16, kind="ExternalInput").ap()
    t["wvT"] = nc.dram_tensor("wvT", [128, NK * 256], BF16, kind="ExternalInput").ap()
    t["wpT"] = nc.dram_tensor("wpT", [128, 2048], BF16, kind="ExternalInput").ap()
    t["cosr"] = nc.dram_tensor("cosr", [128, N], BF16, kind="ExternalInput").ap()
    t["sinr"] = nc.dram_tensor("sinr", [128, N], BF16, kind="ExternalInput").ap()
    t["outp"] = nc.dram_tensor("outp", [N, C], BF16, kind="ExternalOutput").ap()
    with tile.TileContext(nc) as tc:
        for r in range(rep):
            _emit_body(tc, nc, t, uid=f"r{r}" if rep > 1 else "")
    nc.compile()
    return nc


def _make_core_inputs(x, wq, wk, wv, wp, cos, sin, b, hg):
    r0 = CL * hg
    evens = np.concatenate([r0 + D*h + np.arange(0, D, 2) for h in range(HPG)])
    odds = np.concatenate([r0 + D*h + np.arange(1, D, 2) for h in range(HPG)])
    return {
        "xT": _bf16(x[b].T),
        "wqT": _pack_w(wq[np.concatenate([evens, odds])].T),
        "wkT": _pack_w(wk[np.concatenate([evens, odds])].T),
        "wvT": _pack_w(wv[r0:r0+CL].T),
        "wpT": _pack_w(wp[:, r0:r0+CL].T),
        "cosr": _bf16(np.tile(cos.T, (HPG, 1))),
        "sinr": _bf16(np.tile(sin.T, (HPG, 1))),
    }


_CACHE = {}


class _Compiled:
    """Compile once; reusable jitted 8-core SPMD executable (axon/PJRT path)."""

    def __init__(self, nc, n_cores=NCORES):
        import jax
        from jax.sharding import Mesh, PartitionSpec
        from jax.experimental.shard_map import shard_map
        from concourse.bass2jax import (install_neuronx_cc_hook, _bass_exec_p,
                                        partition_id_tensor)
        install_neuronx_cc_hook()
        self.jax = jax
        self.nc = nc
        self.n_cores = n_cores
        in_names, out_names, out_avals, zero_outs = [], [], [], []
        for alloc in nc.m.functions[0].allocations:
            if not isinstance(alloc, mybir.MemoryLocationSet):
                continue
            name = alloc.memorylocations[0].name
            if alloc.kind == "ExternalInput":
                if nc.partition_id_tensor is None or name != nc.partition_id_tensor.name:
                    in_names.append(name)
            elif alloc.kind == "ExternalOutput":
                shape = tuple(alloc.tensor_shape)
                dtype = mybir.dt.np(alloc.dtype)
                out_names.append(name)
                out_avals.append(jax.core.ShapedArray(shape, dtype))
                zero_outs.append(np.zeros(shape, dtype))
        self.in_names, self.out_names = in_names, out_names
        self.out_avals, self.zero_outs = out_avals, zero_outs
        n_params = len(in_names)
        all_in_names = list(in_names) + list(out_names)
        partition_name = nc.partition_id_tensor.name if nc.partition_id_tensor else None
        if partition_name is not None:
            all_in_names.append(partition_name)

        def _body(*args):
            operands = list(args)
            if partition_name is not None:
                operands.append(partition_id_tensor())
            outs = _bass_exec_p.bind(
                *operands, out_avals=tuple(out_avals), in_names=tuple(all_in_names),
                out_names=tuple(out_names), lowering_input_output_aliases=(),
                sim_require_finite=True, sim_require_nnan=True, nc=nc)
            return tuple(outs)

        self.n_params = n_params
        devices = jax.devices()[:n_cores]
        mesh = Mesh(np.asarray(devices), ("core",))
        in_specs = (PartitionSpec("core"),) * (n_params + len(out_names))
        out_specs = (PartitionSpec("core"),) * len(out_names)
        self.fn = jax.jit(
            shard_map(_body, mesh=mesh, in_specs=in_specs, out_specs=out_specs,
                      check_rep=False), keep_unused=True)

    def run(self, in_maps):
        nco = self.n_cores
        concat_in = [np.concatenate([np.asarray(in_maps[c][n]) for c in range(nco)],
                                    axis=0) for n in self.in_names]
        concat_zeros = [np.zeros((nco * z.shape[0], *z.shape[1:]), z.dtype)
                        for z in self.zero_outs]
        outs = self.jax.block_until_ready(self.fn(*concat_in, *concat_zeros))
        return [
            {n: np.asarray(outs[i]).reshape(nco, *self.out_avals[i].shape)[c]
             for i, n in enumerate(self.out_names)}
            for c in range(nco)
        ]


def _get_compiled():
    if "k" not in _CACHE:
        _CACHE["k"] = _Compiled(_build_nc())
    return _CACHE["k"]


def kernel(x, wq, wk, wv, wp, bp, cos, sin, num_heads):
    x = np.asarray(x, dtype=np.float32)
    wq = np.asarray(wq, dtype=np.float32)
    wk = np.asarray(wk, dtype=np.float32)
    wv = np.asarray(wv, dtype=np.float32)
    wp = np.asarray(wp, dtype=np.float32)
    bp = np.asarray(bp, dtype=np.float32)
    cos = np.asarray(cos, dtype=np.float32)
    sin = np.asarray(sin, dtype=np.float32)
    assert int(num_heads) == H, f"kernel hardcodes num_heads={H}"
    assert x.shape == (B, N, C)

    ck = _get_compiled()
    in_maps = [_make_core_inputs(x, wq, wk, wv, wp, cos, sin, c // HPG, c % HPG)
               for c in range(NCORES)]
    results = ck.run(in_maps)
    out = np.zeros((B, N, C), np.float32)
    for c in range(NCORES):
        out[c // HPG] += results[c]["outp"].astype(np.float32)
    out += bp[None, None, :]
    return out


# revision 27
# speedup vs baseline: 1.2837x; 1.2837x over previous
"""Self-contained TRN2 Bass kernel for nn_Attention_26044681683510.

Multi-head attention (B=2, N=2048, C=1024, H=16, rotary, softmax, out-proj),
sharded over 8 NeuronCores as (batch b, head-group hg of 4 heads):
data-parallel on batch, tensor-parallel on heads (column-parallel QKV,
row-parallel out-proj with host-side partial-sum reduction).

v2 dataflow (per core):
  A: Q^T/K^T projections in fp32r (accuracy-critical path) with even/odd
     channel split, rotary on DVE/Pool, DMA rearrange into per-head layouts.
     V projection in fp32r -> stored bf16 with an appended ones column
     (fused softmax-denominator row-sum). All of x and the weights are
     prefetched up front (x both halves resident) so the PE never waits on
     the second-half load.
  B: scores_T = kfp[h].T @ qf (fp32r) -> PSUM; exp on ACT (scale=1/8) ->
     bf16 SBUF; PV = [v|1].T @ exp in bf16 accumulated over k-tiles in PSUM.
     Normalization reads PSUM directly (fast reciprocal + gpsimd
     partition_broadcast + DVE multiply) -> bf16 attn.
  C: partial out-proj attn^T.T @ wpT in bf16 -> (seq, 1024) fp32 PSUM ->
     bf16 partials DMA'd out; host sums the 4 head-group partials per batch
     in fp32 and adds the bias.

QK path stays fp32r (11-bit mantissa) end-to-end; V/softmax/out-proj run in
bf16 (fp32 accumulation) -> ~4.5e-3 scale-relative error vs fp32 reference.
"""
import sys

for _p in ("/opt/trn_rl_repo",):
    if _p not in sys.path:
        sys.path.insert(0, _p)

import numpy as np
import ml_dtypes
import concourse.bass as bass
import concourse.mybir as mybir
import concourse.tile as tile
from concourse import bacc

F32 = mybir.dt.float32
F32R = mybir.dt.float32r
BF16 = mybir.dt.bfloat16
AFT = mybir.ActivationFunctionType

B, N, C, H = 2, 2048, 1024, 16
D = C // H
HPG = 4               # heads per core
CL = HPG * D          # 256 local channels
NK = C // 128         # 8 contraction tiles
NSEQ = N // 128       # 16 seq tiles
VW2 = 72              # padded per-head v block: [v(64) | ones(1) | pad(7)]
NCORES = 8


def _round_fp32r(a):
    b = np.ascontiguousarray(a, dtype=np.float32).view(np.uint32)
    mask = np.uint32((1 << 12) - 1)
    add = (b >> 12) & np.uint32(1)
    return ((b + mask // 2 + add) & ~mask).view(np.float32)


def _bf16(a):
    return np.ascontiguousarray(a, dtype=np.float32).astype(ml_dtypes.bfloat16)


def _pack_w(a):
    """[K*128, M] -> [128, K*M]: k-tile k at columns M*k (single-DMA layout)."""
    K, M = a.shape[0] // 128, a.shape[1]
    return _bf16(a.reshape(K, 128, M).transpose(1, 0, 2).reshape(128, K * M))


def _emit_body(tc, nc, t, uid=""):
    with tc.tile_pool(name=f"cst{uid}", bufs=1) as cst, \
         tc.tile_pool(name=f"qk{uid}", bufs=1) as qkp:
        wp_t = cst.tile([128, 2048], BF16, name=f"wp{uid}")
        nc.sync.dma_start(wp_t[:], t["wpT"][:, :])

        # packed q/k: pair i at columns 2048i; within a pair, head 2i on
        # partitions 0-63 and head 2i+1 on partitions 64-127
        qf_a = qkp.tile([128, 4096], BF16, name=f"qfa{uid}")
        kf_a = qkp.tile([128, 4096], BF16, name=f"kfa{uid}")
        v_sb = [qkp.tile([128, HPG * VW2], BF16, name=f"vsb{m}{uid}") for m in range(NSEQ)]
        attn_f = [qkp.tile([128, 2048], BF16, name=f"af{i}{uid}") for i in range(2)]
        for m in range(NSEQ):
            nc.gpsimd.memset(
                v_sb[m][:].rearrange("p (h w) -> p h w", h=HPG)[:, :, D:VW2], 1.0)

        def q_rows(h):
            return 64 * (h % 2)

        # ---------------- Phase A: QK projections + rotary ----------------
        xp_cm = tc.tile_pool(name=f"xa{uid}", bufs=1)
        xp = xp_cm.__enter__()
        wqkv_cm = tc.tile_pool(name=f"wqkv{uid}", bufs=1)
        wqkv = wqkv_cm.__enter__()
        csp_cm = tc.tile_pool(name=f"cs{uid}", bufs=1)
        csp = csp_cm.__enter__()
        rtp_cm = tc.tile_pool(name=f"rtmp{uid}", bufs=1)
        rtp = rtp_cm.__enter__()
        psA_cm = tc.tile_pool(name=f"psA{uid}", bufs=2, space="PSUM")
        psA = psA_cm.__enter__()

        # host-packed weights: one [128, 8*256] tile per projection, k-tile k
        # at columns 256k (single DMA each)
        wq_t = wqkv.tile([128, NK * 256], BF16, name=f"wq{uid}")
        wk_t = wqkv.tile([128, NK * 256], BF16, name=f"wk{uid}")
        wv_t = wqkv.tile([128, NK * 256], BF16, name=f"wv{uid}")
        # x^T k-tiles, full token range: one [128, 2048] tile per k
        xh = [xp.tile([128, 2048], BF16, name=f"x{k}{uid}") for k in range(NK)]
        cos_f = csp.tile([128, 2048], BF16, name=f"cos{uid}")
        sin_f = csp.tile([128, 2048], BF16, name=f"sin{uid}")
        # issue order ~= arrival order
        nc.sync.dma_start(wq_t[:], t["wqT"][:, :])
        nc.sync.dma_start(wk_t[:], t["wkT"][:, :])
        for k in range(NK):
            nc.sync.dma_start(xh[k][:], t["xT"][128*k:128*(k+1), :])
        nc.sync.dma_start(cos_f[:], t["cosr"][:, :])
        nc.sync.dma_start(sin_f[:], t["sinr"][:, :])
        nc.sync.dma_start(wv_t[:], t["wvT"][:, :])

        for half in range(2):
            rot_t = {(pre, eo): rtp.tile([128, 1024], BF16,
                                         name=f"r{pre}{eo}{half}{uid}",
                                         tag=f"r{pre}{eo}", bufs=2)
                     for pre in ("q", "k") for eo in ("e", "o")}
            for c in range(2):
                gof = 1024 * half + 512 * c
                pp = {}
                for nm, wt, sel in (("qe", wq_t, 0), ("qo", wq_t, 1),
                                    ("ke", wk_t, 0), ("ko", wk_t, 1)):
                    ps = psA.tile([128, 512], F32, name=f"ps_{nm}_{half}_{c}{uid}", tag=f"ps{nm}")
                    for k in range(NK):
                        nc.tensor.matmul(ps[:], wt[:, 256*k+128*sel:256*k+128*(sel+1)],
                                         xh[k][:, gof:gof+512],
                                         start=(k == 0), stop=(k == NK - 1))
                    pp[nm] = ps
                cs = cos_f[:, gof:gof+512]
                sn = sin_f[:, gof:gof+512]
                for pre in ("q", "k"):
                    e_ps, o_ps = pp[pre + "e"], pp[pre + "o"]
                    ta = rtp.tile([128, 512], F32, name=f"ta{pre}{half}{c}{uid}", tag="ta", bufs=2)
                    tb = rtp.tile([128, 512], F32, name=f"tb{pre}{half}{c}{uid}", tag="tb", bufs=2)
                    nc.vector.tensor_mul(ta[:], e_ps[:], cs)
                    nc.vector.tensor_mul(tb[:], o_ps[:], sn)
                    tc2 = rtp.tile([128, 512], F32, name=f"tc{pre}{half}{c}{uid}", tag="tc", bufs=2)
                    td = rtp.tile([128, 512], F32, name=f"td{pre}{half}{c}{uid}", tag="td", bufs=2)
                    nc.vector.tensor_mul(tc2[:], e_ps[:], sn)
                    nc.vector.tensor_mul(td[:], o_ps[:], cs)
                    # full-width rotate at 2x bf16 DVE rate into a per-half
                    # accumulator tile; scatter once per (pre, half) below
                    re = rot_t[(pre, "e")]
                    ro_ = rot_t[(pre, "o")]
                    nc.vector.tensor_sub(re[:, 512*c:512*(c+1)], ta[:], tb[:])
                    nc.vector.tensor_add(ro_[:, 512*c:512*(c+1)], tc2[:], td[:])
                    if c == 1:
                        # plain 2D scatter DMAs: src rows 32h -> dst tile rows
                        # 64*(h%2) (+32 for ro), pair h//2 column block
                        dstt = qf_a if pre == "q" else kf_a
                        hof2 = 1024 * half
                        for h in range(HPG):
                            pc = 2048 * (h // 2) + hof2
                            rw = 64 * (h % 2)
                            nc.sync.dma_start(dstt[rw:rw+32, pc:pc+1024],
                                              re[32*h:32*h+32, :])
                            nc.sync.dma_start(dstt[rw+32:rw+64, pc:pc+1024],
                                              ro_[32*h:32*h+32, :])

        for cm in (psA_cm, rtp_cm, csp_cm):
            cm.__exit__(None, None, None)

        # ------- Phase AB: V projection with head-0 scores/exp overlapped ----
        exp_cm = tc.tile_pool(name=f"ex{uid}", bufs=12, side="right")
        exp_p = exp_cm.__enter__()
        nrp_cm = tc.tile_pool(name=f"nr{uid}", bufs=1, side="right")
        nrp = nrp_cm.__enter__()
        scp_cm = tc.tile_pool(name=f"scp{uid}", bufs=1, space="PSUM", side="right")
        scp = scp_cm.__enter__()
        psV_cm = tc.tile_pool(name=f"psV{uid}", bufs=2, space="PSUM")
        psV = psV_cm.__enter__()

        ex_t = {}

        def emit_scores_exp(h, kt):
            ex = exp_p.tile([128, 2048], BF16, name=f"ex{h}_{kt}{uid}", tag="ex")
            for hv in range(2):
                hof = 1024 * hv
                sc = scp.tile([128, 1024], F32, name=f"sc{h}_{kt}_{hv}{uid}", tag="sc", bufs=2)
                ro = q_rows(h)
                po = 2048 * (h // 2)
                for cc in range(2):
                    nc.tensor.matmul(sc[:, 512*cc:512*(cc+1)],
                                     kf_a[ro:ro+64, po+128*kt:po+128*(kt+1)],
                                     qf_a[ro:ro+64, po+hof+512*cc:po+hof+512*(cc+1)],
                                     start=True, stop=True)
                nc.scalar.activation(ex[:, hof:hof+1024], sc[:], AFT.Exp, scale=0.125)
            ex_t[(h, kt)] = ex

        def emit_pv(h, kt, pv):
            ex = ex_t.pop((h, kt))
            for cc in range(4):
                nc.tensor.matmul(pv[cc // 2][:, 512*(cc % 2):512*(cc % 2 + 1)],
                                 v_sb[kt][:, VW2*h:VW2*h+D+1],
                                 ex[:, 512*cc:512*(cc+1)],
                                 start=(kt == 0), stop=(kt == NSEQ - 1))

        def emit_norm(h, pv):
            ti, off = h // 2, 64 * (h % 2)
            for i in range(2):
                hof = 1024 * i
                rs = nrp.tile([1, 1024], F32, name=f"rs{h}_{i}{uid}", tag=f"rs{i}")
                # custom-DVE ops misread PSUM on HW: bounce through SBUF
                nc.vector.tensor_copy(rs[:], pv[i][64:65, :])
                rsr = nrp.tile([1, 1024], F32, name=f"rsr{h}_{i}{uid}", tag=f"rsr{i}")
                nc.vector.reciprocal_approx_fast(rsr[:], rs[:])
                bc = nrp.tile([64, 1024], F32, name=f"bc{h}_{i}{uid}", tag=f"bc{i}")
                nc.gpsimd.partition_broadcast(bc[:], rsr[:], channels=64)
                nc.vector.tensor_mul(attn_f[ti][off:off+64, hof:hof+1024],
                                     pv[i][0:64, :], bc[:])

        for mt in range(NSEQ):
            psv = psV.tile([128, 256], F32, name=f"psv{mt}{uid}", tag="psv")
            for k in range(NK):
                nc.tensor.matmul(psv[:], xh[k][:, 128*mt:128*(mt+1)],
                                 wv_t[:, 256*k:256*(k+1)],
                                 start=(k == 0), stop=(k == NK - 1))
            vdst = v_sb[mt][:].rearrange("p (h w) -> p h w", h=HPG)[:, :, 0:D]
            vsrc = psv[:].rearrange("p (h d) -> p h d", h=HPG)
            nc.vector.tensor_copy(vdst, vsrc)
            emit_scores_exp(0, mt)

        psV_cm.__exit__(None, None, None)
        wqkv_cm.__exit__(None, None, None)
        xp_cm.__exit__(None, None, None)

        # ---------------- Phase B: attention ----------------
        pvp_cm = tc.tile_pool(name=f"pvp{uid}", bufs=1, space="PSUM")
        pvp = pvp_cm.__enter__()

        for h in range(HPG):
            pv = [pvp.tile([65, 1024], F32, name=f"pv{h}_{i}{uid}", tag=f"pv{i}")
                  for i in range(2)]
            for kt in range(NSEQ):
                if h > 0:
                    emit_scores_exp(h, kt)
                emit_pv(h, kt, pv)
            emit_norm(h, pv)

        for cm in (pvp_cm, scp_cm, nrp_cm, exp_cm):
            cm.__exit__(None, None, None)

        # ---------------- Phase C: partial out-proj ----------------
        with tc.tile_pool(name=f"pop{uid}", bufs=2, space="PSUM") as pop, \
             tc.tile_pool(name=f"ob{uid}", bufs=2) as obp:
            for mt in range(NSEQ):
                po = pop.tile([128, 1024], F32, name=f"po{mt}{uid}", tag="po")
                for k in range(2):
                    for nchunk in range(2):
                        nc.tensor.matmul(po[:, 512*nchunk:512*(nchunk+1)],
                                         attn_f[k][:, 128*mt:128*(mt+1)],
                                         wp_t[:, 1024*k+512*nchunk:1024*k+512*(nchunk+1)],
                                         start=(k == 0), stop=(k == 1))
                ob = obp.tile([128, 1024], BF16, name=f"ob{mt}{uid}", tag=f"ob{mt % 2}", bufs=2)
                if mt % 2 == 0:
                    nc.vector.tensor_copy(ob[:], po[:])
                else:
                    nc.scalar.activation(ob[:], po[:], AFT.Copy)
                nc.sync.dma_start(t["outp"][128*mt:128*(mt+1), :], ob[:])


def _build_nc(rep=1, num_devices=NCORES):
    nc = bacc.Bacc("TRN2", target_bir_lowering=False, debug=False,
                   num_devices=num_devices)
    t = {}
    t["xT"] = nc.dram_tensor("xT", [C, N], BF16, kind="ExternalInput").ap()
    t["wqT"] = nc.dram_tensor("wqT", [128, NK * 256], BF16, kind="ExternalInput").ap()
    t["wkT"] = nc.dram_tensor("wkT", [128, NK * 256], BF16, kind="ExternalInput").ap()
    t["wvT"] = nc.dram_tensor("wvT", [128, NK * 256], BF16, kind="ExternalInput").ap()
    t["wpT"] = nc.dram_tensor("wpT", [128, 2048], BF16, kind="ExternalInput").ap()
    t["cosr"] = nc.dram_tensor("cosr", [128, N], BF16, kind="ExternalInput").ap()
    t["sinr"] = nc.dram_tensor("sinr", [128, N], BF16, kind="ExternalInput").ap()
    t["outp"] = nc.dram_tensor("outp", [N, C], BF16, kind="ExternalOutput").ap()
    with tile.TileContext(nc) as tc:
        for r in range(rep):
            _emit_body(tc, nc, t, uid=f"r{r}" if rep > 1 else "")
    nc.compile()
    return nc


def _make_core_inputs(x, wq, wk, wv, wp, cos, sin, b, hg):
    r0 = CL * hg
    evens = np.concatenate([r0 + D*h + np.arange(0, D, 2) for h in range(HPG)])
    odds = np.concatenate([r0 + D*h + np.arange(1, D, 2) for h in range(HPG)])
    return {
        "xT": _bf16(x[b].T),
        "wqT": _pack_w(wq[np.concatenate([evens, odds])].T),
        "wkT": _pack_w(wk[np.concatenate([evens, odds])].T),
        "wvT": _pack_w(wv[r0:r0+CL].T),
        "wpT": _pack_w(wp[:, r0:r0+CL].T),
        "cosr": _bf16(np.tile(cos.T, (HPG, 1))),
        "sinr": _bf16(np.tile(sin.T, (HPG, 1))),
    }


_CACHE = {}


class _Compiled:
    """Compile once; reusable jitted 8-core SPMD executable (axon/PJRT path)."""

    def __init__(self, nc, n_cores=NCORES):
        import jax
        from jax.sharding import Mesh, PartitionSpec
        from jax.experimental.shard_map import shard_map
        from concourse.bass2jax import (install_neuronx_cc_hook, _bass_exec_p,
                                        partition_id_tensor)
        install_neuronx_cc_hook()
        self.jax = jax
        self.nc = nc
        self.n_cores = n_cores
        in_names, out_names, out_avals, zero_outs = [], [], [], []
        for alloc in nc.m.functions[0].allocations:
            if not isinstance(alloc, mybir.MemoryLocationSet):
                continue
            name = alloc.memorylocations[0].name
            if alloc.kind == "ExternalInput":
                if nc.partition_id_tensor is None or name != nc.partition_id_tensor.name:
                    in_names.append(name)
            elif alloc.kind == "ExternalOutput":
                shape = tuple(alloc.tensor_shape)
                dtype = mybir.dt.np(alloc.dtype)
                out_names.append(name)
                out_avals.append(jax.core.ShapedArray(shape, dtype))
                zero_outs.append(np.zeros(shape, dtype))
        self.in_names, self.out_names = in_names, out_names
        self.out_avals, self.zero_outs = out_avals, zero_outs
        n_params = len(in_names)
        all_in_names = list(in_names) + list(out_names)
        partition_name = nc.partition_id_tensor.name if nc.partition_id_tensor else None
        if partition_name is not None:
            all_in_names.append(partition_name)

        def _body(*args):
            operands = list(args)
            if partition_name is not None:
                operands.append(partition_id_tensor())
            outs = _bass_exec_p.bind(
                *operands, out_avals=tuple(out_avals), in_names=tuple(all_in_names),
                out_names=tuple(out_names), lowering_input_output_aliases=(),
                sim_require_finite=True, sim_require_nnan=True, nc=nc)
            return tuple(outs)

        self.n_params = n_params
        devices = jax.devices()[:n_cores]
        mesh = Mesh(np.asarray(devices), ("core",))
        in_specs = (PartitionSpec("core"),) * (n_params + len(out_names))
        out_specs = (PartitionSpec("core"),) * len(out_names)
        self.fn = jax.jit(
            shard_map(_body, mesh=mesh, in_specs=in_specs, out_specs=out_specs,
                      check_rep=False), keep_unused=True)

    def run(self, in_maps):
        nco = self.n_cores
        concat_in = [np.concatenate([np.asarray(in_maps[c][n]) for c in range(nco)],
                                    axis=0) for n in self.in_names]
        concat_zeros = [np.zeros((nco * z.shape[0], *z.shape[1:]), z.dtype)
                        for z in self.zero_outs]
        outs = self.jax.block_until_ready(self.fn(*concat_in, *concat_zeros))
        return [
            {n: np.asarray(outs[i]).reshape(nco, *self.out_avals[i].shape)[c]
             for i, n in enumerate(self.out_names)}
            for c in range(nco)
        ]


def _get_compiled():
    if "k" not in _CACHE:
        _CACHE["k"] = _Compiled(_build_nc())
    return _CACHE["k"]


def kernel(x, wq, wk, wv, wp, bp, cos, sin, num_heads):
    x = np.asarray(x, dtype=np.float32)
    wq = np.asarray(wq, dtype=np.float32)
    wk = np.asarray(wk, dtype=np.float32)
    wv = np.asarray(wv, dtype=np.float32)
    wp = np.asarray(wp, dtype=np.float32)
    bp = np.asarray(bp, dtype=np.float32)
    cos = np.asarray(cos, dtype=np.float32)
    sin = np.asarray(sin, dtype=np.float32)
    assert int(num_heads) == H, f"kernel hardcodes num_heads={H}"
    assert x.shape == (B, N, C)

    ck = _get_compiled()
    in_maps = [_make_core_inputs(x, wq, wk, wv, wp, cos, sin, c // HPG, c % HPG)
               for c in range(NCORES)]
    results = ck.run(in_maps)
    out = np.zeros((B, N, C), np.float32)
    for c in range(NCORES):
        out[c // HPG] += results[c]["outp"].astype(np.float32)
    out += bp[None, None, :]
    return out


# revision 29
# speedup vs baseline: 1.5287x; 1.1909x over previous
"""Self-contained TRN2 Bass kernel for nn_Attention_26044681683510.

Multi-head attention (B=2, N=2048, C=1024, H=16, rotary, softmax, out-proj),
sharded over 8 NeuronCores as (batch b, head-group hg of 4 heads):
data-parallel on batch, tensor-parallel on heads (column-parallel QKV,
row-parallel out-proj with host-side partial-sum reduction).

All device data is bf16 (fp32 PSUM accumulation); ~1.2e-2 scale-relative
error vs the fp32 reference. DMA count is minimized (HWDGE ring costs
~625ns/DMA): host-packed single-DMA weight tiles, 8 full-token x tiles,
plain-2D-slice scatter DMAs only.

Per-core dataflow:
  A: Q^T/K^T projections (channels on partitions, even/odd split) -> rotary
     on DVE (f32 products, 2 sub/add into bf16 [128,1024] per (pre,half)) ->
     8 plain scatter DMAs per (pre,half) into packed head-pair layouts
     qf_a/kf_a [128, 4096] (pair i at columns 2048i, head pair elements on
     partition halves).
  B: V projection (psV) interleaved with head-0 scores/exp so ACT starts
     early. scores_T = kf.T @ qf as K=64 matmuls at base partition 64*(h%2);
     exp on ACT (scale=1/8 folded) -> bf16 ex [128,2048]; PV = [v|1].T @ ex
     accumulated over k-tiles into per-q-half PSUM tiles (fused ones-column
     row-sum gives softmax denominators). Normalization: denominator row ->
     SBUF copy (custom-DVE ops cannot read PSUM on HW) -> fast reciprocal ->
     gpsimd partition_broadcast -> DVE multiply -> bf16 attn.
  C: partial out-proj attn^T.T @ wpT -> fp32 PSUM -> bf16 partials
     (PSUM->SBUF copies alternate DVE/ACT); host sums the 4 head-group
     partials per batch in fp32 and adds the bias.

Measured on TRN2 (test.py rep-slope): ~197-238us HW exec vs 438us baseline.
"""
import sys

for _p in ("/opt/trn_rl_repo",):
    if _p not in sys.path:
        sys.path.insert(0, _p)

import numpy as np
import ml_dtypes
import concourse.bass as bass
import concourse.mybir as mybir
import concourse.tile as tile
from concourse import bacc

F32 = mybir.dt.float32
F32R = mybir.dt.float32r
BF16 = mybir.dt.bfloat16
AFT = mybir.ActivationFunctionType

B, N, C, H = 2, 2048, 1024, 16
D = C // H
HPG = 4               # heads per core
CL = HPG * D          # 256 local channels
NK = C // 128         # 8 contraction tiles
NSEQ = N // 128       # 16 seq tiles
VW2 = 72              # padded per-head v block: [v(64) | ones(1) | pad(7)]
NCORES = 8


def _round_fp32r(a):
    b = np.ascontiguousarray(a, dtype=np.float32).view(np.uint32)
    mask = np.uint32((1 << 12) - 1)
    add = (b >> 12) & np.uint32(1)
    return ((b + mask // 2 + add) & ~mask).view(np.float32)


def _bf16(a):
    return np.ascontiguousarray(a, dtype=np.float32).astype(ml_dtypes.bfloat16)


def _pack_w(a):
    """[K*128, M] -> [128, K*M]: k-tile k at columns M*k (single-DMA layout)."""
    K, M = a.shape[0] // 128, a.shape[1]
    return _bf16(a.reshape(K, 128, M).transpose(1, 0, 2).reshape(128, K * M))


def _emit_body(tc, nc, t, uid=""):
    with tc.tile_pool(name=f"cst{uid}", bufs=1) as cst, \
         tc.tile_pool(name=f"qk{uid}", bufs=1) as qkp:
        wp_t = cst.tile([128, 2048], BF16, name=f"wp{uid}")
        nc.sync.dma_start(wp_t[:], t["wpT"][:, :])

        # packed q/k: pair i at columns 2048i; within a pair, head 2i on
        # partitions 0-63 and head 2i+1 on partitions 64-127
        qf_a = qkp.tile([128, 4096], BF16, name=f"qfa{uid}")
        kf_a = qkp.tile([128, 4096], BF16, name=f"kfa{uid}")
        v_sb = [qkp.tile([128, HPG * VW2], BF16, name=f"vsb{m}{uid}") for m in range(NSEQ)]
        attn_f = [qkp.tile([128, 2048], BF16, name=f"af{i}{uid}") for i in range(2)]
        for m in range(NSEQ):
            nc.gpsimd.memset(
                v_sb[m][:].rearrange("p (h w) -> p h w", h=HPG)[:, :, D:VW2], 1.0)

        def q_rows(h):
            return 64 * (h % 2)

        # ---------------- Phase A: QK projections + rotary ----------------
        xp_cm = tc.tile_pool(name=f"xa{uid}", bufs=1)
        xp = xp_cm.__enter__()
        wqkv_cm = tc.tile_pool(name=f"wqkv{uid}", bufs=1)
        wqkv = wqkv_cm.__enter__()
        csp_cm = tc.tile_pool(name=f"cs{uid}", bufs=1)
        csp = csp_cm.__enter__()
        rtp_cm = tc.tile_pool(name=f"rtmp{uid}", bufs=1)
        rtp = rtp_cm.__enter__()
        psA_cm = tc.tile_pool(name=f"psA{uid}", bufs=2, space="PSUM")
        psA = psA_cm.__enter__()

        # host-packed weights: one [128, 8*256] tile per projection, k-tile k
        # at columns 256k (single DMA each)
        wq_t = wqkv.tile([128, NK * 256], BF16, name=f"wq{uid}")
        wk_t = wqkv.tile([128, NK * 256], BF16, name=f"wk{uid}")
        wv_t = wqkv.tile([128, NK * 256], BF16, name=f"wv{uid}")
        # x^T k-tiles split by token half: first QK group only waits on half 0
        xh2 = [[xp.tile([128, 1024], BF16, name=f"x{k}_{hf}{uid}")
                for hf in range(2)] for k in range(NK)]
        cos_f = csp.tile([128, 2048], BF16, name=f"cos{uid}")
        sin_f = csp.tile([128, 2048], BF16, name=f"sin{uid}")
        # issue order ~= arrival order
        nc.sync.dma_start(wq_t[:], t["wqT"][:, :])
        for k in range(NK):
            nc.sync.dma_start(xh2[k][0][:], t["xT"][128*k:128*(k+1), 0:1024])
        nc.sync.dma_start(wk_t[:], t["wkT"][:, :])
        for k in range(NK):
            nc.sync.dma_start(xh2[k][1][:], t["xT"][128*k:128*(k+1), 1024:2048])
        nc.sync.dma_start(cos_f[:], t["cosr"][:, :])
        nc.sync.dma_start(sin_f[:], t["sinr"][:, :])
        nc.sync.dma_start(wv_t[:], t["wvT"][:, :])

        for half in range(2):
            rot_t = {(pre, eo): rtp.tile([128, 1024], BF16,
                                         name=f"r{pre}{eo}{half}{uid}",
                                         tag=f"r{pre}{eo}", bufs=2)
                     for pre in ("q", "k") for eo in ("e", "o")}
            for c in range(2):
                gof = 1024 * half + 512 * c
                pp = {}
                for nm, wt, sel in (("qe", wq_t, 0), ("qo", wq_t, 1),
                                    ("ke", wk_t, 0), ("ko", wk_t, 1)):
                    ps = psA.tile([128, 512], F32, name=f"ps_{nm}_{half}_{c}{uid}", tag=f"ps{nm}")
                    for k in range(NK):
                        nc.tensor.matmul(ps[:], wt[:, 256*k+128*sel:256*k+128*(sel+1)],
                                         xh2[k][half][:, 512*c:512*(c+1)],
                                         start=(k == 0), stop=(k == NK - 1))
                    pp[nm] = ps
                cs = cos_f[:, gof:gof+512]
                sn = sin_f[:, gof:gof+512]
                for pre in ("q", "k"):
                    e_ps, o_ps = pp[pre + "e"], pp[pre + "o"]
                    ta = rtp.tile([128, 512], F32, name=f"ta{pre}{half}{c}{uid}", tag="ta", bufs=2)
                    tb = rtp.tile([128, 512], F32, name=f"tb{pre}{half}{c}{uid}", tag="tb", bufs=2)
                    nc.vector.tensor_mul(ta[:], e_ps[:], cs)
                    nc.vector.tensor_mul(tb[:], o_ps[:], sn)
                    tc2 = rtp.tile([128, 512], F32, name=f"tc{pre}{half}{c}{uid}", tag="tc", bufs=2)
                    td = rtp.tile([128, 512], F32, name=f"td{pre}{half}{c}{uid}", tag="td", bufs=2)
                    nc.vector.tensor_mul(tc2[:], e_ps[:], sn)
                    nc.vector.tensor_mul(td[:], o_ps[:], cs)
                    # full-width rotate at 2x bf16 DVE rate into a per-half
                    # accumulator tile; scatter once per (pre, half) below
                    re = rot_t[(pre, "e")]
                    ro_ = rot_t[(pre, "o")]
                    nc.vector.tensor_sub(re[:, 512*c:512*(c+1)], ta[:], tb[:])
                    nc.vector.tensor_add(ro_[:, 512*c:512*(c+1)], tc2[:], td[:])
                    if c == 1:
                        # plain 2D scatter DMAs: src rows 32h -> dst tile rows
                        # 64*(h%2) (+32 for ro), pair h//2 column block
                        dstt = qf_a if pre == "q" else kf_a
                        hof2 = 1024 * half
                        for h in range(HPG):
                            pc = 2048 * (h // 2) + hof2
                            rw = 64 * (h % 2)
                            nc.sync.dma_start(dstt[rw:rw+32, pc:pc+1024],
                                              re[32*h:32*h+32, :])
                            nc.sync.dma_start(dstt[rw+32:rw+64, pc:pc+1024],
                                              ro_[32*h:32*h+32, :])

        for cm in (psA_cm, rtp_cm, csp_cm):
            cm.__exit__(None, None, None)

        # ------- Phase AB: V projection with head-0 scores/exp overlapped ----
        exp_cm = tc.tile_pool(name=f"ex{uid}", bufs=16, side="right")
        exp_p = exp_cm.__enter__()
        nrp_cm = tc.tile_pool(name=f"nr{uid}", bufs=1, side="right")
        nrp = nrp_cm.__enter__()
        scp_cm = tc.tile_pool(name=f"scp{uid}", bufs=1, space="PSUM", side="right")
        scp = scp_cm.__enter__()
        psV_cm = tc.tile_pool(name=f"psV{uid}", bufs=2, space="PSUM")
        psV = psV_cm.__enter__()

        ex_t = {}

        def emit_scores_exp(h, kt):
            ex = exp_p.tile([128, 2048], BF16, name=f"ex{h}_{kt}{uid}", tag="ex")
            for hv in range(2):
                hof = 1024 * hv
                sc = scp.tile([128, 1024], F32, name=f"sc{h}_{kt}_{hv}{uid}", tag="sc", bufs=2)
                ro = q_rows(h)
                po = 2048 * (h // 2)
                for cc in range(2):
                    nc.tensor.matmul(sc[:, 512*cc:512*(cc+1)],
                                     kf_a[ro:ro+64, po+128*kt:po+128*(kt+1)],
                                     qf_a[ro:ro+64, po+hof+512*cc:po+hof+512*(cc+1)],
                                     start=True, stop=True)
                nc.scalar.activation(ex[:, hof:hof+1024], sc[:], AFT.Exp, scale=0.125)
            ex_t[(h, kt)] = ex

        def emit_pv(h, kt, pv):
            ex = ex_t.pop((h, kt))
            for cc in range(4):
                nc.tensor.matmul(pv[cc // 2][:, 512*(cc % 2):512*(cc % 2 + 1)],
                                 v_sb[kt][:, VW2*h:VW2*h+D+1],
                                 ex[:, 512*cc:512*(cc+1)],
                                 start=(kt == 0), stop=(kt == NSEQ - 1))

        def emit_norm(h, pv):
            ti, off = h // 2, 64 * (h % 2)
            for i in range(2):
                hof = 1024 * i
                rs = nrp.tile([1, 1024], F32, name=f"rs{h}_{i}{uid}", tag=f"rs{i}")
                # custom-DVE ops misread PSUM on HW: bounce through SBUF
                nc.vector.tensor_copy(rs[:], pv[i][64:65, :])
                rsr = nrp.tile([1, 1024], F32, name=f"rsr{h}_{i}{uid}", tag=f"rsr{i}")
                nc.vector.reciprocal_approx_fast(rsr[:], rs[:])
                bc = nrp.tile([64, 1024], F32, name=f"bc{h}_{i}{uid}", tag=f"bc{i}")
                nc.gpsimd.partition_broadcast(bc[:], rsr[:], channels=64)
                nc.vector.tensor_mul(attn_f[ti][off:off+64, hof:hof+1024],
                                     pv[i][0:64, :], bc[:])

        for mt in range(NSEQ):
            psv = psV.tile([128, 256], F32, name=f"psv{mt}{uid}", tag="psv")
            for k in range(NK):
                nc.tensor.matmul(psv[:],
                                 xh2[k][mt // 8][:, 128*(mt % 8):128*(mt % 8 + 1)],
                                 wv_t[:, 256*k:256*(k+1)],
                                 start=(k == 0), stop=(k == NK - 1))
            vdst = v_sb[mt][:].rearrange("p (h w) -> p h w", h=HPG)[:, :, 0:D]
            vsrc = psv[:].rearrange("p (h d) -> p h d", h=HPG)
            nc.vector.tensor_copy(vdst, vsrc)
            emit_scores_exp(0, mt)

        psV_cm.__exit__(None, None, None)
        wqkv_cm.__exit__(None, None, None)
        xp_cm.__exit__(None, None, None)

        # ---------------- Phase B: attention ----------------
        pvp_cm = tc.tile_pool(name=f"pvp{uid}", bufs=1, space="PSUM")
        pvp = pvp_cm.__enter__()

        for h in range(HPG):
            pv = [pvp.tile([65, 1024], F32, name=f"pv{h}_{i}{uid}", tag=f"pv{i}")
                  for i in range(2)]
            for kt in range(NSEQ):
                if h > 0:
                    emit_scores_exp(h, kt)
                emit_pv(h, kt, pv)
            emit_norm(h, pv)

        for cm in (pvp_cm, scp_cm, nrp_cm, exp_cm):
            cm.__exit__(None, None, None)

        # ---------------- Phase C: partial out-proj ----------------
        with tc.tile_pool(name=f"pop{uid}", bufs=2, space="PSUM") as pop, \
             tc.tile_pool(name=f"ob{uid}", bufs=2) as obp:
            for mt in range(NSEQ):
                po = pop.tile([128, 1024], F32, name=f"po{mt}{uid}", tag="po")
                for k in range(2):
                    for nchunk in range(2):
                        nc.tensor.matmul(po[:, 512*nchunk:512*(nchunk+1)],
                                         attn_f[k][:, 128*mt:128*(mt+1)],
                                         wp_t[:, 1024*k+512*nchunk:1024*k+512*(nchunk+1)],
                                         start=(k == 0), stop=(k == 1))
                ob = obp.tile([128, 1024], BF16, name=f"ob{mt}{uid}", tag=f"ob{mt % 2}", bufs=2)
                if mt % 2 == 0:
                    nc.vector.tensor_copy(ob[:], po[:])
                else:
                    nc.scalar.activation(ob[:], po[:], AFT.Copy)
                nc.sync.dma_start(t["outp"][128*mt:128*(mt+1), :], ob[:])


def _build_nc(rep=1, num_devices=NCORES):
    nc = bacc.Bacc("TRN2", target_bir_lowering=False, debug=False,
                   num_devices=num_devices)
    t = {}
    t["xT"] = nc.dram_tensor("xT", [C, N], BF16, kind="ExternalInput").ap()
    t["wqT"] = nc.dram_tensor("wqT", [128, NK * 256], BF16, kind="ExternalInput").ap()
    t["wkT"] = nc.dram_tensor("wkT", [128, NK * 256], BF16, kind="ExternalInput").ap()
    t["wvT"] = nc.dram_tensor("wvT", [128, NK * 256], BF16, kind="ExternalInput").ap()
    t["wpT"] = nc.dram_tensor("wpT", [128, 2048], BF16, kind="ExternalInput").ap()
    t["cosr"] = nc.dram_tensor("cosr", [128, N], BF16, kind="ExternalInput").ap()
    t["sinr"] = nc.dram_tensor("sinr", [128, N], BF16, kind="ExternalInput").ap()
    t["outp"] = nc.dram_tensor("outp", [N, C], BF16, kind="ExternalOutput").ap()
    with tile.TileContext(nc) as tc:
        for r in range(rep):
            _emit_body(tc, nc, t, uid=f"r{r}" if rep > 1 else "")
    nc.compile()
    return nc


def _make_core_inputs(x, wq, wk, wv, wp, cos, sin, b, hg):
    r0 = CL * hg
    evens = np.concatenate([r0 + D*h + np.arange(0, D, 2) for h in range(HPG)])
    odds = np.concatenate([r0 + D*h + np.arange(1, D, 2) for h in range(HPG)])
    return {
        "xT": _bf16(x[b].T),
        "wqT": _pack_w(wq[np.concatenate([evens, odds])].T),
        "wkT": _pack_w(wk[np.concatenate([evens, odds])].T),
        "wvT": _pack_w(wv[r0:r0+CL].T),
        "wpT": _pack_w(wp[:, r0:r0+CL].T),
        "cosr": _bf16(np.tile(cos.T, (HPG, 1))),
        "sinr": _bf16(np.tile(sin.T, (HPG, 1))),
    }


_CACHE = {}


class _Compiled:
    """Compile once; reusable jitted 8-core SPMD executable (axon/PJRT path)."""

    def __init__(self, nc, n_cores=NCORES):
        import jax
        from jax.sharding import Mesh, PartitionSpec
        from jax.experimental.shard_map import shard_map
        from concourse.bass2jax import (install_neuronx_cc_hook, _bass_exec_p,
                                        partition_id_tensor)
        install_neuronx_cc_hook()
        self.jax = jax
        self.nc = nc
        self.n_cores = n_cores
        in_names, out_names, out_avals, zero_outs = [], [], [], []
        for alloc in nc.m.functions[0].allocations:
            if not isinstance(alloc, mybir.MemoryLocationSet):
                continue
            name = alloc.memorylocations[0].name
            if alloc.kind == "ExternalInput":
                if nc.partition_id_tensor is None or name != nc.partition_id_tensor.name:
                    in_names.append(name)
            elif alloc.kind == "ExternalOutput":
                shape = tuple(alloc.tensor_shape)
                dtype = mybir.dt.np(alloc.dtype)
                out_names.append(name)
                out_avals.append(jax.core.ShapedArray(shape, dtype))
                zero_outs.append(np.zeros(shape, dtype))
        self.in_names, self.out_names = in_names, out_names
        self.out_avals, self.zero_outs = out_avals, zero_outs
        n_params = len(in_names)
        all_in_names = list(in_names) + list(out_names)
        partition_name = nc.partition_id_tensor.name if nc.partition_id_tensor else None
        if partition_name is not None:
            all_in_names.append(partition_name)

        def _body(*args):
            operands = list(args)
            if partition_name is not None:
                operands.append(partition_id_tensor())
            outs = _bass_exec_p.bind(
                *operands, out_avals=tuple(out_avals), in_names=tuple(all_in_names),
                out_names=tuple(out_names), lowering_input_output_aliases=(),
                sim_require_finite=True, sim_require_nnan=True, nc=nc)
            return tuple(outs)

        self.n_params = n_params
        devices = jax.devices()[:n_cores]
        mesh = Mesh(np.asarray(devices), ("core",))
        in_specs = (PartitionSpec("core"),) * (n_params + len(out_names))
        out_specs = (PartitionSpec("core"),) * len(out_names)
        self.fn = jax.jit(
            shard_map(_body, mesh=mesh, in_specs=in_specs, out_specs=out_specs,
                      check_rep=False), keep_unused=True)

    def run(self, in_maps):
        nco = self.n_cores
        concat_in = [np.concatenate([np.asarray(in_maps[c][n]) for c in range(nco)],
                                    axis=0) for n in self.in_names]
        concat_zeros = [np.zeros((nco * z.shape[0], *z.shape[1:]), z.dtype)
                        for z in self.zero_outs]
        outs = self.jax.block_until_ready(self.fn(*concat_in, *concat_zeros))
        return [
            {n: np.asarray(outs[i]).reshape(nco, *self.out_avals[i].shape)[c]
             for i, n in enumerate(self.out_names)}
            for c in range(nco)
        ]


def _get_compiled():
    if "k" not in _CACHE:
        _CACHE["k"] = _Compiled(_build_nc())
    return _CACHE["k"]


def kernel(x, wq, wk, wv, wp, bp, cos, sin, num_heads):
    x = np.asarray(x, dtype=np.float32)
    wq = np.asarray(wq, dtype=np.float32)
    wk = np.asarray(wk, dtype=np.float32)
    wv = np.asarray(wv, dtype=np.float32)
    wp = np.asarray(wp, dtype=np.float32)
    bp = np.asarray(bp, dtype=np.float32)
    cos = np.asarray(cos, dtype=np.float32)
    sin = np.asarray(sin, dtype=np.float32)
    assert int(num_heads) == H, f"kernel hardcodes num_heads={H}"
    assert x.shape == (B, N, C)

    ck = _get_compiled()
    in_maps = [_make_core_inputs(x, wq, wk, wv, wp, cos, sin, c // HPG, c % HPG)
               for c in range(NCORES)]
    results = ck.run(in_maps)
    out = np.zeros((B, N, C), np.float32)
    for c in range(NCORES):
        out[c // HPG] += results[c]["outp"].astype(np.float32)
    out += bp[None, None, :]
    return out
